# revision 9
# baseline (speedup 1.0000x reference)
"""Self-contained Trainium2 Bass kernel for nn_CausalSelfAttention_18519898980516.

Full inputs:  x [2,2048,4096], Wq/Wk/Wv/Wo [4096,4096]  (torch Linear convention)
Full output:  [2,2048,4096] fp32.

Sharding: tensor-parallel over 4 head-groups (8 heads each) x data-parallel
over the 2 batch elements = 8 NeuronCores. Each core computes
  partial_b,hg = attn(x_b, Wq/Wk/Wv[head-group rows]) @ Wo[:, head-group cols].T
and the host sums the 4 head-group partials per batch element.

All matmuls run single-pass fp16 (PSUM accumulation in fp32). Operand
layout marshalling (transposes + fp16 casts) happens host-side, so the
device program is pure matmul + RoPE elementwise + softmax.

Attention runs in P^T orientation: scores are computed as S^T[k,q] by
swapping the QK matmul operands, so the exp(P) tiles feed the PV matmul
directly as the moving operand (no on-device transposes of P), with row
sums taken by a ones-vector matmul.
"""

import sys
import types

import numpy as np


def _install_axon_ntff_shim():
    """Allow run_bass_kernel_spmd(trace=True) to NTFF-profile under axon when
    the image's antenv lacks axon_hooks. Harmless if never traced."""
    if "antenv.axon_hooks" in sys.modules:
        return
    try:
        from trn_agent_boot.trn_boot import _ntff_profile_via_ctypes
        hook = _ntff_profile_via_ctypes("/opt/axon/libaxon_pjrt.so")
    except Exception:
        return
    mod = types.ModuleType("antenv.axon_hooks")
    mod.get_axon_ntff_profile_hook = lambda: hook
    mod.set_axon_ntff_profile_hook = lambda h: None
    sys.modules["antenv.axon_hooks"] = mod


_install_axon_ntff_shim()

import concourse.bass as bass
import concourse.mybir as mybir
import concourse.bacc as bacc
from concourse import tile

F32 = mybir.dt.float32
F16 = mybir.dt.float16
AF = mybir.ActivationFunctionType
ALU = mybir.AluOpType

NEG = -1.0e9
EXP_BIAS = -4.0  # exp(s*scale + bias); cancels in softmax, keeps exp < fp16 max


def build_program(S=2048, D=4096, HL=8):
    J = HL * 128          # head-group width (8 heads x 128)
    DT = D // 128         # 32 d-tiles
    ST = S // 128         # 16 s-tiles
    SC = S // 512         # 4 s-chunks
    G = S // 512          # attention q groups of 512
    scale = float(128.0 ** -0.5)

    nc = bacc.Bacc("TRN2", target_bir_lowering=False, debug=False)

    # host-marshalled operands (already transposed + fp16)
    xt = nc.dram_tensor("xt", [128, DT, S], F16, kind="ExternalInput").ap()
    wqt = nc.dram_tensor("wqt", [128, DT, J], F16, kind="ExternalInput").ap()
    wkt = nc.dram_tensor("wkt", [128, DT, J], F16, kind="ExternalInput").ap()
    wvt = nc.dram_tensor("wvt", [128, DT, J], F16, kind="ExternalInput").ap()
    wot = nc.dram_tensor("wot", [128, HL, D], F16, kind="ExternalInput").ap()
    cos_d = nc.dram_tensor("cos_t", [128, S], F16, kind="ExternalInput").ap()
    sinn_d = nc.dram_tensor("sinn_t", [128, S], F16, kind="ExternalInput").ap()
    mask_d = nc.dram_tensor("mask01_t", [128, 128], F16, kind="ExternalInput").ap()
    # fp16 partials: host sums 4 head-group partials in fp64; halves the
    # 33.5MB/core output write
    out = nc.dram_tensor("out", [S, D], F16, kind="ExternalOutput").ap()

    with tile.TileContext(nc) as tc:
        with (
            tc.tile_pool(name="persist", bufs=1) as pp,
            tc.tile_pool(name="dram", bufs=1, space="DRAM") as dp,
            tc.tile_pool(name="a_hd", bufs=2) as ahd,
        ):
            maskt = pp.tile([128, 128], F16, tag="maskt")
            ones_m = pp.tile([128, 128], F16, tag="ones_m")
            expb = pp.tile([128, 1], F32, tag="expb")
            cos_s = pp.tile([128, S], F16, tag="cos_s")
            sinn_s = pp.tile([128, S], F16, tag="sinn_s")
            attnT = pp.tile([128, HL, S], F16, tag="attnT")
            nc.vector.memset(ones_m[:, :], 1.0)
            nc.vector.memset(expb[:, :], EXP_BIAS)

            persist_loaded = [False]

            def load_persist():
                # deferred so the first x/W tiles win the DMA queue at t=0
                if persist_loaded[0]:
                    return
                persist_loaded[0] = True
                nc.sync.dma_start(maskt[:, :], mask_d[:, :])
                nc.sync.dma_start(cos_s[:, :], cos_d[:, :])
                nc.sync.dma_start(sinn_s[:, :], sinn_d[:, :])

            # DRAM scratch: rope'd q^T/k^T per head, v in [s, j] layout
            qt_sp = dp.tile([HL, 128, S], F16, name="qt_sp")
            kt_sp = dp.tile([HL, 128, S], F16, name="kt_sp")
            v_sp = dp.tile([ST, 128, J], F16, name="v_sp")

            ev_i = [0]

            def evac(dst, src):
                # round-robin PSUM->SBUF copies (gpsimd cannot read PSUM)
                if ev_i[0] % 2 == 0:
                    nc.scalar.copy(dst, src)
                else:
                    nc.vector.tensor_copy(dst, src)
                ev_i[0] += 1

            preA = {}

            def emit_head_loads(h):
                # each DMA ring is only ~1/16 of aggregate bandwidth, so
                # split each load across two rings
                kth = ahd.tile([128, S], F16, tag="kth")
                qth = ahd.tile([128, S], F16, tag="qth")
                v_h = ahd.tile([128, ST, 128], F16, tag="v_h")
                for sh in range(2):
                    a, b = sh * (S // 2), (sh + 1) * (S // 2)
                    nc.sync.dma_start(kth[:, a:b], kt_sp[h, :, a:b])
                    nc.sync.dma_start(qth[:, a:b], qt_sp[h, :, a:b])
                    a2, b2 = sh * (ST // 2), (sh + 1) * (ST // 2)
                    nc.sync.dma_start(
                        v_h[:, a2:b2, :],
                        v_sp[a2:b2, :, h * 128:(h + 1) * 128].rearrange(
                            "st p hd -> p st hd"))
                return kth, qth, v_h

            # ---------------- Phase P: projections + RoPE -----------------
            with (
                tc.tile_pool(name="p_xc", bufs=2) as pxc,
                tc.tile_pool(name="p_wt", bufs=3) as pwt,
                tc.tile_pool(name="p_wv", bufs=2) as pwv,
                tc.tile_pool(name="p_sb", bufs=3) as psb,
            ):
                def emit_qk(sc, xc, post_jt=None):
                    s0 = sc * 512
                    for jt in range(HL):
                        for t, w_in, spill in (("q", wqt, qt_sp),
                                               ("k", wkt, kt_sp)):
                            wb = pwt.tile([128, DT, 128], F16, tag="wb")
                            for h4 in range(4):
                                d0 = h4 * (DT // 4)
                                d1 = (h4 + 1) * (DT // 4)
                                nc.sync.dma_start(
                                    wb[:, d0:d1, :],
                                    w_in[:, d0:d1, jt * 128:(jt + 1) * 128])
                            load_persist()
                            qp = pps.tile([128, 512], F32, tag="qp")
                            DQ = DT // 8
                            for d in range(DT):
                                nc.tensor.matmul(
                                    qp[:, :], wb[:, d, :],
                                    xc[d // DQ][:, d % DQ, :],
                                    start=(d == 0), stop=(d == DT - 1),
                                    skip_group_check=True)
                            # rope: qf = q*cos + swap_halves(q)*sinn
                            qs = psb.tile([128, 512], F16, tag="qs")
                            nc.scalar.copy(qs[:, :], qp[:, :])
                            sq = psb.tile([128, 512], F16, tag="sq")
                            nc.gpsimd.tensor_copy(sq[0:64, :], qs[64:128, :])
                            nc.gpsimd.tensor_copy(sq[64:128, :], qs[0:64, :])
                            m1 = psb.tile([128, 512], F16, tag="m1")
                            nc.vector.tensor_tensor(
                                m1[:, :], qs[:, :], cos_s[:, s0:s0 + 512],
                                ALU.mult)
                            rp = psb.tile([128, 512], F16, tag="rp")
                            nc.vector.tensor_tensor(
                                rp[:, :], sq[:, :], sinn_s[:, s0:s0 + 512],
                                ALU.mult)
                            qf = psb.tile([128, 512], F16, tag="qf")
                            nc.vector.tensor_tensor(
                                qf[:, :], m1[:, :], rp[:, :], ALU.add)
                            nc.sync.dma_start(
                                spill[jt, :, s0:s0 + 512], qf[:, :])
                        if post_jt is not None:
                            post_jt(jt)

                def emit_v(sc, xc):
                    # v projection in [s, j] orientation (x^T stationary)
                    for jc in range(4):
                        j0 = jc * 256
                        wvb = pwv.tile([128, DT, 256], F16, tag="wvb")
                        for h4 in range(4):
                            d0 = h4 * (DT // 4)
                            d1 = (h4 + 1) * (DT // 4)
                            nc.sync.dma_start(
                                wvb[:, d0:d1, :], wvt[:, d0:d1, j0:j0 + 256])
                        for st in range(4):
                            vp = pvp.tile([128, 256], F32, tag="vp")
                            DQ = DT // 8
                            for d in range(DT):
                                nc.tensor.matmul(
                                    vp[:, :],
                                    xc[d // DQ][:, d % DQ,
                                                st * 128:(st + 1) * 128],
                                    wvb[:, d, :],
                                    start=(d == 0), stop=(d == DT - 1),
                                    skip_group_check=True)
                            vs = psb.tile([128, 256], F16, tag="vs")
                            evac(vs[:, :], vp[:, :])
                            nc.sync.dma_start(
                                v_sp[sc * 4 + st, :, j0:j0 + 256], vs[:, :])

                with (
                    tc.tile_pool(name="p_ps", bufs=4, space="PSUM") as pps,
                    tc.tile_pool(name="p_vp", bufs=3, space="PSUM") as pvp,
                ):
                    for sc in range(SC):
                        s0 = sc * 512
                        # 8 independent tiles: the first matmuls gate on 1/8
                        # of the chunk load instead of the whole 2.1 MB
                        xc = [pxc.tile([128, DT // 8, 512], F16,
                                       tag=f"xc{q8}", name=f"xc{q8}")
                              for q8 in range(8)]
                        # split each tile load along the d dim so the first
                        # matmuls gate on a 128-256KB DMA, not 512KB on 1 ring
                        nsp = 4 if sc == 0 else 2
                        step = (DT // 8) // nsp
                        for q8 in range(8):
                            d0 = q8 * (DT // 8)
                            for s2 in range(nsp):
                                nc.sync.dma_start(
                                    xc[q8][:, s2 * step:(s2 + 1) * step, :],
                                    xt[:, d0 + s2 * step:d0 + (s2 + 1) * step,
                                       s0:s0 + 512])
                        if sc < SC - 1:
                            emit_qk(sc, xc)
                            emit_v(sc, xc)
                        else:
                            # last chunk: finish v first so the first heads'
                            # attention inputs can stream during the q/k tail
                            emit_v(sc, xc)

                            def prefetch(jt):
                                # after BOTH q and k of this head-tile spilled
                                if jt in (0, 1):
                                    preA[jt] = emit_head_loads(jt)

                            emit_qk(sc, xc, post_jt=prefetch)

            # ---------------- Phase A: attention per head -----------------
            with (
                tc.tile_pool(name="a_pt", bufs=2) as apt,
                tc.tile_pool(name="a_sb", bufs=3) as asb,
                tc.tile_pool(name="w_wt", bufs=2) as wwt,
                tc.tile_pool(name="w_sb", bufs=3) as wsb,
            ):
                PIPE = 3  # QK runs this many k-tiles ahead of rs/PV on PE

                # prefetch the first Wo chunk while attention runs
                def load_wob(dc):
                    wob = wwt.tile([128, HL, 512], F16, tag="wob")
                    for j2 in range(4):
                        nc.sync.dma_start(
                            wob[:, 2 * j2:2 * j2 + 2, :],
                            wot[:, 2 * j2:2 * j2 + 2,
                                dc * 512:(dc + 1) * 512])
                    return wob

                wob0 = load_wob(0)

                # normalize of group g runs on DVE behind group g+1's causal
                # masks, so the slow reciprocal never blocks the mask->exp->PV
                # chain the PE is waiting on
                pending_norm = [None]

                def flush_norm():
                    if pending_norm[0] is None:
                        return
                    ph, pq0, prs, pot = pending_norm[0]
                    pending_norm[0] = None
                    rcp = asb.tile([128, 512], F32, tag="rcp")
                    nc.vector.reciprocal(rcp[:, :], prs[:, :])
                    nc.vector.tensor_tensor(
                        attnT[:, ph, pq0:pq0 + 512], pot[:, :], rcp[:, :],
                        ALU.mult)

                with (
                    tc.tile_pool(name="a_sc", bufs=4, space="PSUM") as asc,
                    tc.tile_pool(name="a_ot", bufs=2, space="PSUM") as aot,
                    tc.tile_pool(name="a_rs", bufs=2, space="PSUM") as ars,
                ):
                    for h in range(HL):
                        if h in preA:
                            kth, qth, v_h = preA.pop(h)
                        else:
                            kth, qth, v_h = emit_head_loads(h)
                        # double-buffer: next head's tiles stream during this
                        # head's compute (ahd pool bufs=2)
                        if h + 1 < HL and h + 1 not in preA:
                            preA[h + 1] = emit_head_loads(h + 1)
                        for g in range(G):
                            q0 = g * 512
                            nkt = 4 * (g + 1)
                            ept = apt.tile([128, 16, 512], F16, tag="ept")
                            rs = ars.tile([128, 512], F32, tag="rs")
                            ot = aot.tile([128, 512], F32, tag="ot")

                            def emit_pv(kt):
                                qo = max(0, kt - 4 * g) * 128
                                nc.tensor.matmul(
                                    ot[:, qo:], v_h[:, kt, :],
                                    ept[:, kt, qo:],
                                    start=(kt == 0), stop=(kt == nkt - 1),
                                    skip_group_check=True)
                                # rowsum pre-broadcast to all partitions:
                                # every output row is the same column sum
                                nc.tensor.matmul(
                                    rs[:, qo:], ones_m[:, :],
                                    ept[:, kt, qo:],
                                    start=(kt == 0), stop=(kt == nkt - 1),
                                    skip_group_check=True)

                            for kt in range(nkt):
                                ql = kt - 4 * g
                                qoff = max(0, ql) * 128
                                sp = asc.tile([128, 512], F32, tag="sp")
                                nc.tensor.matmul(
                                    sp[:, qoff:],
                                    kth[:, kt * 128:(kt + 1) * 128],
                                    qth[:, q0 + qoff:q0 + 512],
                                    start=True, stop=True,
                                    skip_group_check=True)
                                nc.scalar.activation(
                                    ept[:, kt, qoff:], sp[:, qoff:], AF.Exp,
                                    bias=expb[:, :], scale=scale)
                                if ql >= 0:
                                    # zero the strict-causal triangle after
                                    # exp (raw diag scores stay in fp16
                                    # range), keeping the DVE hop off the
                                    # QK->exp->PSUM-recycle chain
                                    nc.vector.tensor_tensor(
                                        ept[:, kt, qoff:qoff + 128],
                                        ept[:, kt, qoff:qoff + 128],
                                        maskt[:, :], ALU.mult)
                                if kt == nkt - 1:
                                    flush_norm()
                                if kt >= PIPE:
                                    emit_pv(kt - PIPE)
                            for kt in range(max(0, nkt - PIPE), nkt):
                                emit_pv(kt)
                            pending_norm[0] = (h, q0, rs, ot)
                    flush_norm()

                # ---------------- Phase W: out = attn_out @ wo.T ----------
                with tc.tile_pool(name="w_ps", bufs=4, space="PSUM") as wps:
                    for dc in range(D // 512):
                        if dc == 0:
                            wob = wob0
                        else:
                            wob = load_wob(dc)
                        for st in range(ST):
                            ps = wps.tile([128, 512], F32, tag="wp")
                            for jt in range(HL):
                                nc.tensor.matmul(
                                    ps[:, :],
                                    attnT[:, jt, st * 128:(st + 1) * 128],
                                    wob[:, jt, :],
                                    start=(jt == 0), stop=(jt == HL - 1),
                                    skip_group_check=True)
                            og = wsb.tile([128, 512], F16, tag="og")
                            evac(og[:, :], ps[:, :])
                            nc.sync.dma_start(
                                out[st * 128:(st + 1) * 128,
                                    dc * 512:(dc + 1) * 512],
                                og[:, :])

    nc.compile()
    return nc


def make_consts(S):
    """Host-side constant tensors (cos/sinn/mask)."""
    HD = 128
    inv_freq = (1.0 / (10000.0 ** (np.arange(0, HD, 2, dtype=np.float64) / HD)))
    pos = np.arange(S, dtype=np.float64)
    freqs = pos[:, None] * inv_freq[None, :]
    emb = np.concatenate([freqs, freqs], axis=-1)          # [S, 128]
    cos_t = np.ascontiguousarray(np.cos(emb).T.astype(np.float16))  # [128, S]
    sin = np.sin(emb).T                                     # [128, S]
    # swap_halves(q)[p] = q[p+64] (p<64) else q[p-64]; q_rot = q*cos + sq*sinn
    # where sinn[p] = -sin[p] for p<64 else +sin[p]
    sinn = sin.copy()
    sinn[:64] *= -1.0
    sinn_t = np.ascontiguousarray(sinn.astype(np.float16))
    mask01 = np.where(np.arange(128)[:, None] > np.arange(128)[None, :],
                      np.float16(0.0), np.float16(1.0))     # 0 where k>q
    return {"cos_t": cos_t, "sinn_t": sinn_t, "mask01_t": mask01}


_NC_CACHE = {}


def _get_program():
    if "nc" not in _NC_CACHE:
        _NC_CACHE["nc"] = build_program(S=2048, D=4096, HL=8)
    return _NC_CACHE["nc"]


LAST_EXEC_TIME_NS = None
LAST_RESULTS = None


def kernel(x, Wq, Wk, Wv, Wo):
    """Full-input entry point. Shards across 8 NeuronCores, returns [B,S,D]."""
    import os
    from concourse import bass_utils

    global LAST_EXEC_TIME_NS, LAST_RESULTS
    x = np.asarray(x, dtype=np.float32)
    B, S, D = x.shape
    NG = 4  # head groups
    J = D // NG
    DT = D // 128
    HL = J // 128

    consts = make_consts(S)
    nc = _get_program()

    def tile_T(a):  # [R, D] fp -> [128, DT, R] fp16, out[p, dt, r] = a[r, dt*128+p]
        R = a.shape[0]
        return np.ascontiguousarray(
            a.T.reshape(DT, 128, R).transpose(1, 0, 2).astype(np.float16))

    xT = [tile_T(x[b]) for b in range(B)]
    in_maps = []
    for hg in range(NG):
        wq_t = tile_T(np.asarray(Wq[hg * J:(hg + 1) * J, :], dtype=np.float32))
        wk_t = tile_T(np.asarray(Wk[hg * J:(hg + 1) * J, :], dtype=np.float32))
        wv_t = tile_T(np.asarray(Wv[hg * J:(hg + 1) * J, :], dtype=np.float32))
        # wot[p, jt, dout] = Wo[dout, hg*J + jt*128 + p]
        wo_s = np.asarray(Wo[:, hg * J:(hg + 1) * J], dtype=np.float32)  # [D, J]
        wo_t = np.ascontiguousarray(
            wo_s.T.reshape(HL, 128, D).transpose(1, 0, 2).astype(np.float16))
        for b in range(B):
            m = {"xt": xT[b], "wqt": wq_t, "wkt": wk_t, "wvt": wv_t,
                 "wot": wo_t}
            m.update(consts)
            in_maps.append(m)

    trace = bool(int(os.environ.get("BASS_KERNEL_TRACE", "0")))
    res = bass_utils.run_bass_kernel_spmd(
        nc, in_maps, core_ids=list(range(NG * B)), trace=trace
    )
    LAST_EXEC_TIME_NS = res.exec_time_ns
    LAST_RESULTS = res

    out = np.zeros((B, S, D), dtype=np.float64)
    for hg in range(NG):
        for b in range(B):
            out[b] += res.results[hg * B + b]["out"].astype(np.float64)
    return out.astype(np.float32)



# revision 10
# speedup vs baseline: 1.0364x; 1.0364x over previous
"""Self-contained Trainium2 Bass kernel for nn_CausalSelfAttention_18519898980516.

Full inputs:  x [2,2048,4096], Wq/Wk/Wv/Wo [4096,4096]  (torch Linear convention)
Full output:  [2,2048,4096] fp32.

Sharding: tensor-parallel over 4 head-groups (8 heads each) x data-parallel
over the 2 batch elements = 8 NeuronCores. Each core computes
  partial_b,hg = attn(x_b, Wq/Wk/Wv[head-group rows]) @ Wo[:, head-group cols].T
and the host sums the 4 head-group partials per batch element.

All matmuls run single-pass fp16 (PSUM accumulation in fp32). Operand
layout marshalling (transposes + fp16 casts) happens host-side, so the
device program is pure matmul + RoPE elementwise + softmax.

Attention runs in P^T orientation: scores are computed as S^T[k,q] by
swapping the QK matmul operands, so the exp(P) tiles feed the PV matmul
directly as the moving operand (no on-device transposes of P), with row
sums taken by a ones-vector matmul.
"""

import sys
import types

import numpy as np


def _install_axon_ntff_shim():
    """Allow run_bass_kernel_spmd(trace=True) to NTFF-profile under axon when
    the image's antenv lacks axon_hooks. Harmless if never traced."""
    if "antenv.axon_hooks" in sys.modules:
        return
    try:
        from trn_agent_boot.trn_boot import _ntff_profile_via_ctypes
        hook = _ntff_profile_via_ctypes("/opt/axon/libaxon_pjrt.so")
    except Exception:
        return
    mod = types.ModuleType("antenv.axon_hooks")
    mod.get_axon_ntff_profile_hook = lambda: hook
    mod.set_axon_ntff_profile_hook = lambda h: None
    sys.modules["antenv.axon_hooks"] = mod


_install_axon_ntff_shim()

import concourse.bass as bass
import concourse.mybir as mybir
import concourse.bacc as bacc
from concourse import tile

F32 = mybir.dt.float32
F16 = mybir.dt.float16
AF = mybir.ActivationFunctionType
ALU = mybir.AluOpType

NEG = -1.0e9
EXP_BIAS = -4.0  # exp(s*scale + bias); cancels in softmax, keeps exp < fp16 max


def build_program(S=2048, D=4096, HL=8):
    J = HL * 128          # head-group width (8 heads x 128)
    DT = D // 128         # 32 d-tiles
    ST = S // 128         # 16 s-tiles
    SC = S // 512         # 4 s-chunks
    G = S // 512          # attention q groups of 512
    scale = float(128.0 ** -0.5)

    nc = bacc.Bacc("TRN2", target_bir_lowering=False, debug=False)

    # host-marshalled operands (already transposed + fp16)
    xt = nc.dram_tensor("xt", [128, DT, S], F16, kind="ExternalInput").ap()
    wqt = nc.dram_tensor("wqt", [128, DT, J], F16, kind="ExternalInput").ap()
    wkt = nc.dram_tensor("wkt", [128, DT, J], F16, kind="ExternalInput").ap()
    wvt = nc.dram_tensor("wvt", [128, DT, J], F16, kind="ExternalInput").ap()
    wot = nc.dram_tensor("wot", [128, HL, D], F16, kind="ExternalInput").ap()
    cos_d = nc.dram_tensor("cos_t", [128, S], F16, kind="ExternalInput").ap()
    sinn_d = nc.dram_tensor("sinn_t", [128, S], F16, kind="ExternalInput").ap()
    mask_d = nc.dram_tensor("mask01_t", [128, 128], F16, kind="ExternalInput").ap()
    # fp16 partials: host sums in fp64; halves the output write
    out = nc.dram_tensor("out", [S, D], F16, kind="ExternalOutput").ap()

    with tile.TileContext(nc) as tc:
        with (
            tc.tile_pool(name="persist", bufs=1) as pp,
            tc.tile_pool(name="dram", bufs=1, space="DRAM") as dp,
            tc.tile_pool(name="a_hd", bufs=2) as ahd,
        ):
            maskt = pp.tile([128, 128], F16, tag="maskt")
            ones_m = pp.tile([128, 128], F16, tag="ones_m")
            expb = pp.tile([128, 1], F32, tag="expb")
            cos_s = pp.tile([128, S], F16, tag="cos_s")
            sinn_s = pp.tile([128, S], F16, tag="sinn_s")
            attnT = pp.tile([128, HL, S], F16, tag="attnT")
            nc.vector.memset(ones_m[:, :], 1.0)
            nc.vector.memset(expb[:, :], EXP_BIAS)

            persist_loaded = [False]

            def load_persist():
                # deferred so the first x/W tiles win the DMA queue at t=0
                if persist_loaded[0]:
                    return
                persist_loaded[0] = True
                nc.sync.dma_start(maskt[:, :], mask_d[:, :])
                nc.sync.dma_start(cos_s[:, :], cos_d[:, :])
                nc.sync.dma_start(sinn_s[:, :], sinn_d[:, :])

            # DRAM scratch: rope'd q^T/k^T per head, v in [s, j] layout
            qt_sp = dp.tile([HL, 128, S], F16, name="qt_sp")
            kt_sp = dp.tile([HL, 128, S], F16, name="kt_sp")
            v_sp = dp.tile([ST, 128, J], F16, name="v_sp")

            ev_i = [0]

            def evac(dst, src):
                # round-robin PSUM->SBUF copies (gpsimd cannot read PSUM)
                if ev_i[0] % 2 == 0:
                    nc.scalar.copy(dst, src)
                else:
                    nc.vector.tensor_copy(dst, src)
                ev_i[0] += 1

            preA = {}

            def emit_head_loads(h):
                # each DMA ring is only ~1/16 of aggregate bandwidth, so
                # split each load across two rings
                kth = ahd.tile([128, S], F16, tag="kth")
                qth = ahd.tile([128, S], F16, tag="qth")
                v_h = ahd.tile([128, ST, 128], F16, tag="v_h")
                for sh in range(2):
                    a, b = sh * (S // 2), (sh + 1) * (S // 2)
                    nc.sync.dma_start(kth[:, a:b], kt_sp[h, :, a:b])
                    nc.sync.dma_start(qth[:, a:b], qt_sp[h, :, a:b])
                    a2, b2 = sh * (ST // 2), (sh + 1) * (ST // 2)
                    nc.sync.dma_start(
                        v_h[:, a2:b2, :],
                        v_sp[a2:b2, :, h * 128:(h + 1) * 128].rearrange(
                            "st p hd -> p st hd"))
                return kth, qth, v_h

            # ---------------- Phase P: projections + RoPE -----------------
            with (
                tc.tile_pool(name="p_xc", bufs=2) as pxc,
                tc.tile_pool(name="p_wt", bufs=3) as pwt,
                tc.tile_pool(name="p_wv", bufs=2) as pwv,
                tc.tile_pool(name="p_sb", bufs=3) as psb,
            ):
                wb0_box = [None]

                def preload_wb0():
                    wb = pwt.tile([128, DT, 128], F16, tag="wb", name="wb0")
                    for h8 in range(8):
                        d0 = h8 * (DT // 8)
                        d1 = (h8 + 1) * (DT // 8)
                        nc.sync.dma_start(
                            wb[:, d0:d1, :], wqt[:, d0:d1, 0:128])
                    wb0_box[0] = wb

                def emit_qk(sc, xc, post_jt=None):
                    s0 = sc * 512
                    for jt in range(HL):
                        for t, w_in, spill in (("q", wqt, qt_sp),
                                               ("k", wkt, kt_sp)):
                            if sc == 0 and jt == 0 and t == "q":
                                wb = wb0_box[0]
                            else:
                                wb = pwt.tile([128, DT, 128], F16, tag="wb")
                                for h2 in range(2):
                                    d0 = h2 * (DT // 2)
                                    d1 = (h2 + 1) * (DT // 2)
                                    nc.sync.dma_start(
                                        wb[:, d0:d1, :],
                                        w_in[:, d0:d1,
                                             jt * 128:(jt + 1) * 128])
                            load_persist()
                            qp = pps.tile([128, 512], F32, tag="qp")
                            DQ = DT // 8
                            for d in range(DT):
                                nc.tensor.matmul(
                                    qp[:, :], wb[:, d, :],
                                    xc[d // DQ][:, d % DQ, :],
                                    start=(d == 0), stop=(d == DT - 1),
                                    skip_group_check=True)
                            # rope: qf = q*cos + swap_halves(q)*sinn
                            qs = psb.tile([128, 512], F16, tag="qs")
                            nc.scalar.copy(qs[:, :], qp[:, :])
                            sq = psb.tile([128, 512], F16, tag="sq")
                            nc.gpsimd.tensor_copy(sq[0:64, :], qs[64:128, :])
                            nc.gpsimd.tensor_copy(sq[64:128, :], qs[0:64, :])
                            m1 = psb.tile([128, 512], F16, tag="m1")
                            nc.vector.tensor_tensor(
                                m1[:, :], qs[:, :], cos_s[:, s0:s0 + 512],
                                ALU.mult)
                            rp = psb.tile([128, 512], F16, tag="rp")
                            nc.vector.tensor_tensor(
                                rp[:, :], sq[:, :], sinn_s[:, s0:s0 + 512],
                                ALU.mult)
                            qf = psb.tile([128, 512], F16, tag="qf")
                            nc.vector.tensor_tensor(
                                qf[:, :], m1[:, :], rp[:, :], ALU.add)
                            nc.sync.dma_start(
                                spill[jt, :, s0:s0 + 512], qf[:, :])
                        if post_jt is not None:
                            post_jt(jt)

                def emit_v(sc, xc):
                    # v projection in [s, j] orientation (x^T stationary)
                    for jc in range(4):
                        j0 = jc * 256
                        wvb = pwv.tile([128, DT, 256], F16, tag="wvb")
                        for h2 in range(2):
                            d0 = h2 * (DT // 2)
                            d1 = (h2 + 1) * (DT // 2)
                            nc.sync.dma_start(
                                wvb[:, d0:d1, :], wvt[:, d0:d1, j0:j0 + 256])
                        for st in range(4):
                            vp = pvp.tile([128, 256], F32, tag="vp")
                            DQ = DT // 8
                            for d in range(DT):
                                nc.tensor.matmul(
                                    vp[:, :],
                                    xc[d // DQ][:, d % DQ,
                                                st * 128:(st + 1) * 128],
                                    wvb[:, d, :],
                                    start=(d == 0), stop=(d == DT - 1),
                                    skip_group_check=True)
                            vs = psb.tile([128, 256], F16, tag="vs")
                            evac(vs[:, :], vp[:, :])
                            nc.sync.dma_start(
                                v_sp[sc * 4 + st, :, j0:j0 + 256], vs[:, :])

                with (
                    tc.tile_pool(name="p_ps", bufs=4, space="PSUM") as pps,
                    tc.tile_pool(name="p_vp", bufs=3, space="PSUM") as pvp,
                ):
                    for sc in range(SC):
                        s0 = sc * 512
                        # 8 independent tiles: the first matmuls gate on 1/8
                        # of the chunk load instead of the whole 2.1 MB
                        xc = [pxc.tile([128, DT // 8, 512], F16,
                                       tag=f"xc{q8}", name=f"xc{q8}")
                              for q8 in range(8)]
                        if sc == 0:
                            preload_wb0()
                        for q8 in range(8):
                            d0 = q8 * (DT // 8)
                            d1 = (q8 + 1) * (DT // 8)
                            if sc == 0 and q8 == 0:
                                for s4 in range(4):
                                    nc.sync.dma_start(
                                        xc[q8][:, s4:s4 + 1, :],
                                        xt[:, d0 + s4:d0 + s4 + 1,
                                           s0:s0 + 512])
                            else:
                                nc.sync.dma_start(
                                    xc[q8][:, :, :],
                                    xt[:, d0:d1, s0:s0 + 512])
                        if sc < SC - 1:
                            emit_qk(sc, xc)
                            emit_v(sc, xc)
                        else:
                            # last chunk: finish v first so the first heads'
                            # attention inputs can stream during the q/k tail
                            emit_v(sc, xc)

                            def prefetch(jt):
                                # after BOTH q and k of this head-tile spilled
                                if jt in (0, 1):
                                    preA[jt] = emit_head_loads(jt)

                            emit_qk(sc, xc, post_jt=prefetch)

            # ---------------- Phase A: attention per head -----------------
            with (
                tc.tile_pool(name="a_pt", bufs=2) as apt,
                tc.tile_pool(name="a_sb", bufs=3) as asb,
                tc.tile_pool(name="w_wt", bufs=2) as wwt,
                tc.tile_pool(name="w_sb", bufs=3) as wsb,
            ):
                PIPE = 3  # QK runs this many k-tiles ahead of rs/PV on PE

                # prefetch the first Wo chunk while attention runs
                wob0 = wwt.tile([128, HL, 512], F16, tag="wob")
                nc.sync.dma_start(wob0[:, :, :], wot[:, :, 0:512])

                # normalize of group g runs on DVE behind group g+1's causal
                # masks, so the slow reciprocal never blocks the mask->exp->PV
                # chain the PE is waiting on
                pending_norm = [None]

                def flush_norm():
                    if pending_norm[0] is None:
                        return
                    ph, pq0, prs, pot = pending_norm[0]
                    pending_norm[0] = None
                    rcp = asb.tile([128, 512], F32, tag="rcp")
                    nc.vector.reciprocal(rcp[:, :], prs[:, :])
                    nc.vector.tensor_tensor(
                        attnT[:, ph, pq0:pq0 + 512], pot[:, :], rcp[:, :],
                        ALU.mult)

                with (
                    tc.tile_pool(name="a_sc", bufs=4, space="PSUM") as asc,
                    tc.tile_pool(name="a_ot", bufs=2, space="PSUM") as aot,
                    tc.tile_pool(name="a_rs", bufs=2, space="PSUM") as ars,
                ):
                    for h in range(HL):
                        if h in preA:
                            kth, qth, v_h = preA.pop(h)
                        else:
                            kth, qth, v_h = emit_head_loads(h)
                        for g in range(G):
                            q0 = g * 512
                            nkt = 4 * (g + 1)
                            ept = apt.tile([128, 16, 512], F16, tag="ept")
                            rs = ars.tile([128, 512], F32, tag="rs")
                            ot = aot.tile([128, 512], F32, tag="ot")

                            def emit_pv(kt):
                                qo = max(0, kt - 4 * g) * 128
                                nc.tensor.matmul(
                                    ot[:, qo:], v_h[:, kt, :],
                                    ept[:, kt, qo:],
                                    start=(kt == 0), stop=(kt == nkt - 1),
                                    skip_group_check=True)
                                # rowsum pre-broadcast to all partitions:
                                # every output row is the same column sum
                                nc.tensor.matmul(
                                    rs[:, qo:], ones_m[:, :],
                                    ept[:, kt, qo:],
                                    start=(kt == 0), stop=(kt == nkt - 1),
                                    skip_group_check=True)

                            for kt in range(nkt):
                                ql = kt - 4 * g
                                qoff = max(0, ql) * 128
                                sp = asc.tile([128, 512], F32, tag="sp")
                                nc.tensor.matmul(
                                    sp[:, qoff:],
                                    kth[:, kt * 128:(kt + 1) * 128],
                                    qth[:, q0 + qoff:q0 + 512],
                                    start=True, stop=True,
                                    skip_group_check=True)
                                nc.scalar.activation(
                                    ept[:, kt, qoff:], sp[:, qoff:], AF.Exp,
                                    bias=expb[:, :], scale=scale)
                                if ql >= 0:
                                    # zero the strict-causal triangle after
                                    # exp (raw diag scores stay in fp16
                                    # range), keeping the DVE hop off the
                                    # QK->exp->PSUM-recycle chain
                                    nc.vector.tensor_tensor(
                                        ept[:, kt, qoff:qoff + 128],
                                        ept[:, kt, qoff:qoff + 128],
                                        maskt[:, :], ALU.mult)
                                if kt == nkt - 1:
                                    flush_norm()
                                if kt >= PIPE:
                                    emit_pv(kt - PIPE)
                            for kt in range(max(0, nkt - PIPE), nkt):
                                emit_pv(kt)
                            pending_norm[0] = (h, q0, rs, ot)
                    flush_norm()

                # ---------------- Phase W: out = attn_out @ wo.T ----------
                with tc.tile_pool(name="w_ps", bufs=4, space="PSUM") as wps:
                    for dc in range(D // 512):
                        if dc == 0:
                            wob = wob0
                        else:
                            wob = wwt.tile([128, HL, 512], F16, tag="wob")
                            nc.sync.dma_start(
                                wob[:, :, :],
                                wot[:, :, dc * 512:(dc + 1) * 512])
                        for st in range(ST):
                            ps = wps.tile([128, 512], F32, tag="wp")
                            for jt in range(HL):
                                nc.tensor.matmul(
                                    ps[:, :],
                                    attnT[:, jt, st * 128:(st + 1) * 128],
                                    wob[:, jt, :],
                                    start=(jt == 0), stop=(jt == HL - 1),
                                    skip_group_check=True)
                            og = wsb.tile([128, 512], F16, tag="og")
                            evac(og[:, :], ps[:, :])
                            nc.sync.dma_start(
                                out[st * 128:(st + 1) * 128,
                                    dc * 512:(dc + 1) * 512],
                                og[:, :])

    nc.compile()
    return nc


def make_consts(S):
    """Host-side constant tensors (cos/sinn/mask)."""
    HD = 128
    inv_freq = (1.0 / (10000.0 ** (np.arange(0, HD, 2, dtype=np.float64) / HD)))
    pos = np.arange(S, dtype=np.float64)
    freqs = pos[:, None] * inv_freq[None, :]
    emb = np.concatenate([freqs, freqs], axis=-1)          # [S, 128]
    cos_t = np.ascontiguousarray(np.cos(emb).T.astype(np.float16))  # [128, S]
    sin = np.sin(emb).T                                     # [128, S]
    # swap_halves(q)[p] = q[p+64] (p<64) else q[p-64]; q_rot = q*cos + sq*sinn
    # where sinn[p] = -sin[p] for p<64 else +sin[p]
    sinn = sin.copy()
    sinn[:64] *= -1.0
    sinn_t = np.ascontiguousarray(sinn.astype(np.float16))
    mask01 = np.where(np.arange(128)[:, None] > np.arange(128)[None, :],
                      np.float16(0.0), np.float16(1.0))     # 0 where k>q
    return {"cos_t": cos_t, "sinn_t": sinn_t, "mask01_t": mask01}


_NC_CACHE = {}


def _get_program():
    if "nc" not in _NC_CACHE:
        _NC_CACHE["nc"] = build_program(S=2048, D=4096, HL=8)
    return _NC_CACHE["nc"]


LAST_EXEC_TIME_NS = None
LAST_RESULTS = None


def kernel(x, Wq, Wk, Wv, Wo):
    """Full-input entry point. Shards across 8 NeuronCores, returns [B,S,D]."""
    import os
    from concourse import bass_utils

    global LAST_EXEC_TIME_NS, LAST_RESULTS
    x = np.asarray(x, dtype=np.float32)
    B, S, D = x.shape
    NG = 4  # head groups
    J = D // NG
    DT = D // 128
    HL = J // 128

    consts = make_consts(S)
    nc = _get_program()

    def tile_T(a):  # [R, D] fp -> [128, DT, R] fp16, out[p, dt, r] = a[r, dt*128+p]
        R = a.shape[0]
        return np.ascontiguousarray(
            a.T.reshape(DT, 128, R).transpose(1, 0, 2).astype(np.float16))

    xT = [tile_T(x[b]) for b in range(B)]
    in_maps = []
    for hg in range(NG):
        wq_t = tile_T(np.asarray(Wq[hg * J:(hg + 1) * J, :], dtype=np.float32))
        wk_t = tile_T(np.asarray(Wk[hg * J:(hg + 1) * J, :], dtype=np.float32))
        wv_t = tile_T(np.asarray(Wv[hg * J:(hg + 1) * J, :], dtype=np.float32))
        # wot[p, jt, dout] = Wo[dout, hg*J + jt*128 + p]
        wo_s = np.asarray(Wo[:, hg * J:(hg + 1) * J], dtype=np.float32)  # [D, J]
        wo_t = np.ascontiguousarray(
            wo_s.T.reshape(HL, 128, D).transpose(1, 0, 2).astype(np.float16))
        for b in range(B):
            m = {"xt": xT[b], "wqt": wq_t, "wkt": wk_t, "wvt": wv_t,
                 "wot": wo_t}
            m.update(consts)
            in_maps.append(m)

    trace = bool(int(os.environ.get("BASS_KERNEL_TRACE", "0")))
    res = bass_utils.run_bass_kernel_spmd(
        nc, in_maps, core_ids=list(range(NG * B)), trace=trace
    )
    LAST_EXEC_TIME_NS = res.exec_time_ns
    LAST_RESULTS = res

    out = np.zeros((B, S, D), dtype=np.float64)
    for hg in range(NG):
        for b in range(B):
            out[b] += res.results[hg * B + b]["out"].astype(np.float64)
    return out.astype(np.float32)



# revision 11
# speedup vs baseline: 1.0412x; 1.0046x over previous
"""Self-contained Trainium2 Bass kernel for nn_CausalSelfAttention_18519898980516.

Full inputs:  x [2,2048,4096], Wq/Wk/Wv/Wo [4096,4096]  (torch Linear convention)
Full output:  [2,2048,4096] fp32.

Sharding: tensor-parallel over 4 head-groups (8 heads each) x data-parallel
over the 2 batch elements = 8 NeuronCores. Each core computes
  partial_b,hg = attn(x_b, Wq/Wk/Wv[head-group rows]) @ Wo[:, head-group cols].T
and the host sums the 4 head-group partials per batch element.

All matmuls run single-pass fp16 (PSUM accumulation in fp32). Operand
layout marshalling (transposes + fp16 casts) happens host-side, so the
device program is pure matmul + RoPE elementwise + softmax.

Attention runs in P^T orientation: scores are computed as S^T[k,q] by
swapping the QK matmul operands, so the exp(P) tiles feed the PV matmul
directly as the moving operand (no on-device transposes of P), with row
sums taken by a ones-vector matmul.
"""

import sys
import types

import numpy as np


def _install_axon_ntff_shim():
    """Allow run_bass_kernel_spmd(trace=True) to NTFF-profile under axon when
    the image's antenv lacks axon_hooks. Harmless if never traced."""
    if "antenv.axon_hooks" in sys.modules:
        return
    try:
        from trn_agent_boot.trn_boot import _ntff_profile_via_ctypes
        hook = _ntff_profile_via_ctypes("/opt/axon/libaxon_pjrt.so")
    except Exception:
        return
    mod = types.ModuleType("antenv.axon_hooks")
    mod.get_axon_ntff_profile_hook = lambda: hook
    mod.set_axon_ntff_profile_hook = lambda h: None
    sys.modules["antenv.axon_hooks"] = mod


_install_axon_ntff_shim()

import concourse.bass as bass
import concourse.mybir as mybir
import concourse.bacc as bacc
from concourse import tile

F32 = mybir.dt.float32
F16 = mybir.dt.float16
AF = mybir.ActivationFunctionType
ALU = mybir.AluOpType

NEG = -1.0e9
EXP_BIAS = -4.0  # exp(s*scale + bias); cancels in softmax, keeps exp < fp16 max


def build_program(S=2048, D=4096, HL=8):
    J = HL * 128          # head-group width (8 heads x 128)
    DT = D // 128         # 32 d-tiles
    ST = S // 128         # 16 s-tiles
    SC = S // 512         # 4 s-chunks
    G = S // 512          # attention q groups of 512
    scale = float(128.0 ** -0.5)

    nc = bacc.Bacc("TRN2", target_bir_lowering=False, debug=False)

    # host-marshalled operands (already transposed + fp16)
    xt = nc.dram_tensor("xt", [128, DT, S], F16, kind="ExternalInput").ap()
    wqt = nc.dram_tensor("wqt", [128, DT, J], F16, kind="ExternalInput").ap()
    wkt = nc.dram_tensor("wkt", [128, DT, J], F16, kind="ExternalInput").ap()
    wvt = nc.dram_tensor("wvt", [128, DT, J], F16, kind="ExternalInput").ap()
    wot = nc.dram_tensor("wot", [128, HL, D], F16, kind="ExternalInput").ap()
    cos_d = nc.dram_tensor("cos_t", [128, S], F16, kind="ExternalInput").ap()
    sinn_d = nc.dram_tensor("sinn_t", [128, S], F16, kind="ExternalInput").ap()
    mask_d = nc.dram_tensor("mask01_t", [128, 128], F16, kind="ExternalInput").ap()
    # fp16 partials: host sums in fp64; halves the output write
    out = nc.dram_tensor("out", [S, D], F16, kind="ExternalOutput").ap()

    with tile.TileContext(nc) as tc:
        with (
            tc.tile_pool(name="persist", bufs=1) as pp,
            tc.tile_pool(name="dram", bufs=1, space="DRAM") as dp,
            tc.tile_pool(name="a_hd", bufs=2) as ahd,
        ):
            maskt = pp.tile([128, 128], F16, tag="maskt")
            ones_m = pp.tile([128, 128], F16, tag="ones_m")
            expb = pp.tile([128, 1], F32, tag="expb")
            cos_s = pp.tile([128, S], F16, tag="cos_s")
            sinn_s = pp.tile([128, S], F16, tag="sinn_s")
            attnT = pp.tile([128, HL, S], F16, tag="attnT")
            nc.vector.memset(ones_m[:, :], 1.0)
            nc.vector.memset(expb[:, :], EXP_BIAS)

            persist_loaded = [False]

            def load_persist():
                # deferred so the first x/W tiles win the DMA queue at t=0
                if persist_loaded[0]:
                    return
                persist_loaded[0] = True
                nc.sync.dma_start(maskt[:, :], mask_d[:, :])
                nc.sync.dma_start(cos_s[:, :], cos_d[:, :])
                nc.sync.dma_start(sinn_s[:, :], sinn_d[:, :])

            # DRAM scratch: rope'd q^T/k^T per head, v in [s, j] layout
            qt_sp = dp.tile([HL, 128, S], F16, name="qt_sp")
            kt_sp = dp.tile([HL, 128, S], F16, name="kt_sp")
            v_sp = dp.tile([ST, 128, J], F16, name="v_sp")

            ev_i = [0]

            def evac(dst, src):
                # round-robin PSUM->SBUF copies (gpsimd cannot read PSUM)
                if ev_i[0] % 2 == 0:
                    nc.scalar.copy(dst, src)
                else:
                    nc.vector.tensor_copy(dst, src)
                ev_i[0] += 1

            preA = {}

            def emit_head_loads(h):
                # each DMA ring is only ~1/16 of aggregate bandwidth, so
                # split each load across two rings
                kth = ahd.tile([128, S], F16, tag="kth")
                qth = ahd.tile([128, S], F16, tag="qth")
                v_h = ahd.tile([128, ST, 128], F16, tag="v_h")
                for sh in range(2):
                    a, b = sh * (S // 2), (sh + 1) * (S // 2)
                    nc.sync.dma_start(kth[:, a:b], kt_sp[h, :, a:b])
                    nc.sync.dma_start(qth[:, a:b], qt_sp[h, :, a:b])
                    a2, b2 = sh * (ST // 2), (sh + 1) * (ST // 2)
                    nc.sync.dma_start(
                        v_h[:, a2:b2, :],
                        v_sp[a2:b2, :, h * 128:(h + 1) * 128].rearrange(
                            "st p hd -> p st hd"))
                return kth, qth, v_h

            # ---------------- Phase P: projections + RoPE -----------------
            with (
                tc.tile_pool(name="p_xc", bufs=2) as pxc,
                tc.tile_pool(name="p_wt", bufs=3) as pwt,
                tc.tile_pool(name="p_wv", bufs=2) as pwv,
                tc.tile_pool(name="p_sb", bufs=3) as psb,
            ):
                wb0_box = [None]

                def preload_wb0():
                    wb = pwt.tile([128, DT, 128], F16, tag="wb", name="wb0")
                    for h8 in range(8):
                        d0 = h8 * (DT // 8)
                        d1 = (h8 + 1) * (DT // 8)
                        nc.sync.dma_start(
                            wb[:, d0:d1, :], wqt[:, d0:d1, 0:128])
                    wb0_box[0] = wb

                def emit_qk(sc, xc, post_jt=None):
                    s0 = sc * 512
                    for jt in range(HL):
                        for t, w_in, spill in (("q", wqt, qt_sp),
                                               ("k", wkt, kt_sp)):
                            if sc == 0 and jt == 0 and t == "q":
                                wb = wb0_box[0]
                            else:
                                wb = pwt.tile([128, DT, 128], F16, tag="wb")
                                for h2 in range(2):
                                    d0 = h2 * (DT // 2)
                                    d1 = (h2 + 1) * (DT // 2)
                                    nc.sync.dma_start(
                                        wb[:, d0:d1, :],
                                        w_in[:, d0:d1,
                                             jt * 128:(jt + 1) * 128])
                            load_persist()
                            qp = pps.tile([128, 512], F32, tag="qp")
                            DQ = DT // 8
                            for d in range(DT):
                                nc.tensor.matmul(
                                    qp[:, :], wb[:, d, :],
                                    xc[d // DQ][:, d % DQ, :],
                                    start=(d == 0), stop=(d == DT - 1),
                                    skip_group_check=True)
                            # rope: qf = q*cos + swap_halves(q)*sinn
                            qs = psb.tile([128, 512], F16, tag="qs")
                            nc.scalar.copy(qs[:, :], qp[:, :])
                            sq = psb.tile([128, 512], F16, tag="sq")
                            nc.gpsimd.tensor_copy(sq[0:64, :], qs[64:128, :])
                            nc.gpsimd.tensor_copy(sq[64:128, :], qs[0:64, :])
                            m1 = psb.tile([128, 512], F16, tag="m1")
                            nc.vector.tensor_tensor(
                                m1[:, :], qs[:, :], cos_s[:, s0:s0 + 512],
                                ALU.mult)
                            rp = psb.tile([128, 512], F16, tag="rp")
                            nc.vector.tensor_tensor(
                                rp[:, :], sq[:, :], sinn_s[:, s0:s0 + 512],
                                ALU.mult)
                            qf = psb.tile([128, 512], F16, tag="qf")
                            nc.vector.tensor_tensor(
                                qf[:, :], m1[:, :], rp[:, :], ALU.add)
                            nc.sync.dma_start(
                                spill[jt, :, s0:s0 + 512], qf[:, :])
                        if post_jt is not None:
                            post_jt(jt)

                def emit_v(sc, xc):
                    # v projection in [s, j] orientation (x^T stationary)
                    for jc in range(4):
                        j0 = jc * 256
                        wvb = pwv.tile([128, DT, 256], F16, tag="wvb")
                        for h2 in range(2):
                            d0 = h2 * (DT // 2)
                            d1 = (h2 + 1) * (DT // 2)
                            nc.sync.dma_start(
                                wvb[:, d0:d1, :], wvt[:, d0:d1, j0:j0 + 256])
                        for st in range(4):
                            vp = pvp.tile([128, 256], F32, tag="vp")
                            DQ = DT // 8
                            for d in range(DT):
                                nc.tensor.matmul(
                                    vp[:, :],
                                    xc[d // DQ][:, d % DQ,
                                                st * 128:(st + 1) * 128],
                                    wvb[:, d, :],
                                    start=(d == 0), stop=(d == DT - 1),
                                    skip_group_check=True)
                            vs = psb.tile([128, 256], F16, tag="vs")
                            evac(vs[:, :], vp[:, :])
                            nc.sync.dma_start(
                                v_sp[sc * 4 + st, :, j0:j0 + 256], vs[:, :])

                with (
                    tc.tile_pool(name="p_ps", bufs=4, space="PSUM") as pps,
                    tc.tile_pool(name="p_vp", bufs=3, space="PSUM") as pvp,
                ):
                    for sc in range(SC):
                        s0 = sc * 512
                        # 8 independent tiles: the first matmuls gate on 1/8
                        # of the chunk load instead of the whole 2.1 MB
                        xc = [pxc.tile([128, DT // 8, 512], F16,
                                       tag=f"xc{q8}", name=f"xc{q8}")
                              for q8 in range(8)]
                        if sc == 0:
                            preload_wb0()
                        for q8 in range(8):
                            d0 = q8 * (DT // 8)
                            d1 = (q8 + 1) * (DT // 8)
                            if sc == 0 and q8 == 0:
                                for s4 in range(4):
                                    nc.sync.dma_start(
                                        xc[q8][:, s4:s4 + 1, :],
                                        xt[:, d0 + s4:d0 + s4 + 1,
                                           s0:s0 + 512])
                            else:
                                nc.sync.dma_start(
                                    xc[q8][:, :, :],
                                    xt[:, d0:d1, s0:s0 + 512])
                        if sc < SC - 1:
                            emit_qk(sc, xc)
                            emit_v(sc, xc)
                        else:
                            # last chunk: finish v first so the first heads'
                            # attention inputs can stream during the q/k tail
                            emit_v(sc, xc)

                            def prefetch(jt):
                                # after BOTH q and k of this head-tile spilled
                                if jt in (0, 1):
                                    preA[jt] = emit_head_loads(jt)

                            emit_qk(sc, xc, post_jt=prefetch)

            # ---------------- Phase A: attention per head -----------------
            with (
                tc.tile_pool(name="a_pt", bufs=2) as apt,
                tc.tile_pool(name="a_sb", bufs=3) as asb,
                tc.tile_pool(name="w_wt", bufs=2) as wwt,
                tc.tile_pool(name="w_sb", bufs=3) as wsb,
            ):
                PIPE = 3  # QK runs this many k-tiles ahead of rs/PV on PE

                # prefetch the first Wo chunk while attention runs
                def load_wob(dc):
                    wob = wwt.tile([128, HL, 512], F16, tag="wob",
                                   name="wob")
                    for j2 in range(4):
                        nc.sync.dma_start(
                            wob[:, 2 * j2:2 * j2 + 2, :],
                            wot[:, 2 * j2:2 * j2 + 2,
                                dc * 512:(dc + 1) * 512])
                    return wob

                wob0 = load_wob(0)

                # normalize of group g runs on DVE behind group g+1's causal
                # masks, so the slow reciprocal never blocks the mask->exp->PV
                # chain the PE is waiting on
                pending_norm = [None]

                def flush_norm():
                    if pending_norm[0] is None:
                        return
                    ph, pq0, prs, pot = pending_norm[0]
                    pending_norm[0] = None
                    rcp = asb.tile([128, 512], F32, tag="rcp")
                    nc.vector.reciprocal(rcp[:, :], prs[:, :])
                    nc.vector.tensor_tensor(
                        attnT[:, ph, pq0:pq0 + 512], pot[:, :], rcp[:, :],
                        ALU.mult)

                with (
                    tc.tile_pool(name="a_sc", bufs=4, space="PSUM") as asc,
                    tc.tile_pool(name="a_ot", bufs=2, space="PSUM") as aot,
                    tc.tile_pool(name="a_rs", bufs=2, space="PSUM") as ars,
                ):
                    for h in range(HL):
                        if h in preA:
                            kth, qth, v_h = preA.pop(h)
                        else:
                            kth, qth, v_h = emit_head_loads(h)
                        if h + 1 < HL and h + 1 not in preA:
                            preA[h + 1] = emit_head_loads(h + 1)
                        for g in range(G):
                            q0 = g * 512
                            nkt = 4 * (g + 1)
                            ept = apt.tile([128, 16, 512], F16, tag="ept")
                            rs = ars.tile([128, 512], F32, tag="rs")
                            ot = aot.tile([128, 512], F32, tag="ot")

                            def emit_pv(kt):
                                qo = max(0, kt - 4 * g) * 128
                                nc.tensor.matmul(
                                    ot[:, qo:], v_h[:, kt, :],
                                    ept[:, kt, qo:],
                                    start=(kt == 0), stop=(kt == nkt - 1),
                                    skip_group_check=True)
                                # rowsum pre-broadcast to all partitions:
                                # every output row is the same column sum
                                nc.tensor.matmul(
                                    rs[:, qo:], ones_m[:, :],
                                    ept[:, kt, qo:],
                                    start=(kt == 0), stop=(kt == nkt - 1),
                                    skip_group_check=True)

                            for kt in range(nkt):
                                ql = kt - 4 * g
                                qoff = max(0, ql) * 128
                                sp = asc.tile([128, 512], F32, tag="sp")
                                nc.tensor.matmul(
                                    sp[:, qoff:],
                                    kth[:, kt * 128:(kt + 1) * 128],
                                    qth[:, q0 + qoff:q0 + 512],
                                    start=True, stop=True,
                                    skip_group_check=True)
                                nc.scalar.activation(
                                    ept[:, kt, qoff:], sp[:, qoff:], AF.Exp,
                                    bias=expb[:, :], scale=scale)
                                if ql >= 0:
                                    # zero the strict-causal triangle after
                                    # exp (raw diag scores stay in fp16
                                    # range), keeping the DVE hop off the
                                    # QK->exp->PSUM-recycle chain
                                    nc.vector.tensor_tensor(
                                        ept[:, kt, qoff:qoff + 128],
                                        ept[:, kt, qoff:qoff + 128],
                                        maskt[:, :], ALU.mult)
                                if kt == nkt - 1:
                                    flush_norm()
                                if kt >= PIPE:
                                    emit_pv(kt - PIPE)
                            for kt in range(max(0, nkt - PIPE), nkt):
                                emit_pv(kt)
                            pending_norm[0] = (h, q0, rs, ot)
                    flush_norm()

                # ---------------- Phase W: out = attn_out @ wo.T ----------
                with tc.tile_pool(name="w_ps", bufs=4, space="PSUM") as wps:
                    for dc in range(D // 512):
                        if dc == 0:
                            wob = wob0
                        else:
                            wob = load_wob(dc)
                        for st in range(ST):
                            ps = wps.tile([128, 512], F32, tag="wp")
                            for jt in range(HL):
                                nc.tensor.matmul(
                                    ps[:, :],
                                    attnT[:, jt, st * 128:(st + 1) * 128],
                                    wob[:, jt, :],
                                    start=(jt == 0), stop=(jt == HL - 1),
                                    skip_group_check=True)
                            og = wsb.tile([128, 512], F16, tag="og")
                            evac(og[:, :], ps[:, :])
                            nc.sync.dma_start(
                                out[st * 128:(st + 1) * 128,
                                    dc * 512:(dc + 1) * 512],
                                og[:, :])

    nc.compile()
    return nc


def make_consts(S):
    """Host-side constant tensors (cos/sinn/mask)."""
    HD = 128
    inv_freq = (1.0 / (10000.0 ** (np.arange(0, HD, 2, dtype=np.float64) / HD)))
    pos = np.arange(S, dtype=np.float64)
    freqs = pos[:, None] * inv_freq[None, :]
    emb = np.concatenate([freqs, freqs], axis=-1)          # [S, 128]
    cos_t = np.ascontiguousarray(np.cos(emb).T.astype(np.float16))  # [128, S]
    sin = np.sin(emb).T                                     # [128, S]
    # swap_halves(q)[p] = q[p+64] (p<64) else q[p-64]; q_rot = q*cos + sq*sinn
    # where sinn[p] = -sin[p] for p<64 else +sin[p]
    sinn = sin.copy()
    sinn[:64] *= -1.0
    sinn_t = np.ascontiguousarray(sinn.astype(np.float16))
    mask01 = np.where(np.arange(128)[:, None] > np.arange(128)[None, :],
                      np.float16(0.0), np.float16(1.0))     # 0 where k>q
    return {"cos_t": cos_t, "sinn_t": sinn_t, "mask01_t": mask01}


_NC_CACHE = {}


def _get_program():
    if "nc" not in _NC_CACHE:
        _NC_CACHE["nc"] = build_program(S=2048, D=4096, HL=8)
    return _NC_CACHE["nc"]


LAST_EXEC_TIME_NS = None
LAST_RESULTS = None


def kernel(x, Wq, Wk, Wv, Wo):
    """Full-input entry point. Shards across 8 NeuronCores, returns [B,S,D]."""
    import os
    from concourse import bass_utils

    global LAST_EXEC_TIME_NS, LAST_RESULTS
    x = np.asarray(x, dtype=np.float32)
    B, S, D = x.shape
    NG = 4  # head groups
    J = D // NG
    DT = D // 128
    HL = J // 128

    consts = make_consts(S)
    nc = _get_program()

    def tile_T(a):  # [R, D] fp -> [128, DT, R] fp16, out[p, dt, r] = a[r, dt*128+p]
        R = a.shape[0]
        return np.ascontiguousarray(
            a.T.reshape(DT, 128, R).transpose(1, 0, 2).astype(np.float16))

    xT = [tile_T(x[b]) for b in range(B)]
    in_maps = []
    for hg in range(NG):
        wq_t = tile_T(np.asarray(Wq[hg * J:(hg + 1) * J, :], dtype=np.float32))
        wk_t = tile_T(np.asarray(Wk[hg * J:(hg + 1) * J, :], dtype=np.float32))
        wv_t = tile_T(np.asarray(Wv[hg * J:(hg + 1) * J, :], dtype=np.float32))
        # wot[p, jt, dout] = Wo[dout, hg*J + jt*128 + p]
        wo_s = np.asarray(Wo[:, hg * J:(hg + 1) * J], dtype=np.float32)  # [D, J]
        wo_t = np.ascontiguousarray(
            wo_s.T.reshape(HL, 128, D).transpose(1, 0, 2).astype(np.float16))
        for b in range(B):
            m = {"xt": xT[b], "wqt": wq_t, "wkt": wk_t, "wvt": wv_t,
                 "wot": wo_t}
            m.update(consts)
            in_maps.append(m)

    trace = bool(int(os.environ.get("BASS_KERNEL_TRACE", "0")))
    res = bass_utils.run_bass_kernel_spmd(
        nc, in_maps, core_ids=list(range(NG * B)), trace=trace
    )
    LAST_EXEC_TIME_NS = res.exec_time_ns
    LAST_RESULTS = res

    out = np.zeros((B, S, D), dtype=np.float64)
    for hg in range(NG):
        for b in range(B):
            out[b] += res.results[hg * B + b]["out"].astype(np.float64)
    return out.astype(np.float32)

